# revision 3
# baseline (speedup 1.0000x reference)
"""Contrastive cosine-similarity MSE loss kernel for Trainium2 (8 cores).

Math (reference): scores_n = <a_n, b_n> / (||a_n|| * ||b_n||);
loss = mean((scores - labels)^2) over N=8192 rows, D=1024.

Per core (1024 rows): 24 row-stat reductions (8 blocks x {dot, nb, na})
over [128, 1024] fp16 blocks, spread over four engines so each stays
under the ~12 us DMA window (4.2 MB fp16 at ~354 GB/s):
  - ScalarE: 8 na stats as Square activations with accum_out
    (~1.4 us each incl. the accumulator read).
  - VectorE: 13 products (dot_c = a*b, and most nb_c = b*b) in fp16
    2x_1P mode (~0.65 us each), 4 segmented PSUM reduces, and the
    cosine+MSE tail.
  - GpSimd: 3 nb products (~2.2 us each) - otherwise idle silicon.
  - TensorE: folds each [128, 2048] product tile into its pair's PSUM
    bank ([128, 4, 128], slots dot/nb interleaved) with 8 identity-
    stationary accumulating matmuls; warmed from the preamble so the
    HAM clock gate (cold = 1.2 GHz) opens before the first fold.
All DMA descriptor programming lives on Sync (wire order a0a, b0a,
a0b, b0b, a1, b1, a2, b2, a3, b3 so block 0 lands ~1.4 us early);
ScalarE only programs the tiny prepacked-labels DMA so its ACT stream
(table load first, then squares) starts as soon as a-data lands.
Labels arrive host-prepacked as [128, 8] fp32 matching the stats
layout - no on-device transpose.

Embeddings are downcast to fp16 on the host (cosine is scale-invariant
to first order; measured end-to-end loss error ~1e-7). All reductions
accumulate in fp32.

Sharding: data-parallel over rows; core c handles rows
[c*1024, (c+1)*1024). Tiles are [128 partitions x 2048] fp16 where
partition p holds rows (2p, 2p+1) of a 256-row block (4KB-contiguous
DRAM runs -> fat DMA packets). Block c = 2t+j has row(p) = 256t+2p+j.
The final 128-partition partial SSE is reduced to [1,1] with a
ones-matmul; host sums the 8 per-core scalars.
"""

import numpy as np

import concourse.bacc as bacc
import concourse.bass as bass
import concourse.tile as tile
from concourse import mybir
from concourse.bass_utils import run_bass_kernel_spmd
from concourse.masks import make_identity
from concourse.vector_clock import ScopedClock


class _LeanTileContext(tile.TileContext):
    """TileContext with a minimal kernel epilogue.

    The stock epilogue is drain + all-engine butterfly + semaphore
    clear + second butterfly. For this single-shot kernel we only need
    the drain (all DMA queues complete, so the output is in DRAM before
    the NEFF retires); engines may retire their streams independently."""

    def _drain_and_barrier(self, tick_clock, wait_clock):
        drain_inst = self.nc.sync.drain()
        wait_clock.add_sem_waits(
            drain_inst.ins, ScopedClock({None: tick_clock.global_clock})
        )
        popped = self.nc._tile_sem_poison_stack.pop()
        assert popped is self._sem_poison


N, D = 8192, 1024
N_CORES = 8
ROWS = N // N_CORES  # rows per core
P = 128  # SBUF partitions
RPT = 2 * P  # rows per tile (2 per partition)
NTILES = ROWS // RPT  # 4
NBLK = 2 * NTILES  # 128-row blocks (tile t, half j -> c = 2t+j)
KCH = 8  # fold chunks per 1024-col product
PE_WARM = 8  # warmup matmuls to open the HAM clock gate early

# nb_c ownership: GpSimd takes these blocks' b*b product; the rest and
# all dot_c go to VectorE. na_c is always ScalarE.
GP_NB = (0, 2, 4)

_cache = {}


def _build():
    nc = bacc.Bacc("TRN2", target_bir_lowering=False, debug=False)

    f32 = mybir.dt.float32
    f16 = mybir.dt.float16
    a = nc.dram_tensor("a", [ROWS, D], f16, kind="ExternalInput")
    b = nc.dram_tensor("b", [ROWS, D], f16, kind="ExternalInput")
    lab = nc.dram_tensor("lab_t", [P, NBLK], f32, kind="ExternalInput")
    out = nc.dram_tensor("out", [1, 1], f32, kind="ExternalOutput")

    with _LeanTileContext(nc) as tc:
        with (
            tc.tile_pool(name="io", bufs=NTILES) as io_pool,
            tc.tile_pool(name="prod", bufs=4) as prod_pool,
            tc.tile_pool(name="sq", bufs=2) as sq_pool,
            tc.tile_pool(name="fold", bufs=3, space="PSUM") as fold_pool,
            tc.tile_pool(name="psa", bufs=1, space="PSUM") as psa_pool,
            tc.tile_pool(name="stats", bufs=1) as st_pool,
        ):
            # --- upfront DMA: all on Sync, interleaved a/b ------------
            # Wire order = program order on the single HWDGE queue, so
            # tile 0 goes as half-tiles (a0a, b0a first) to open block 0
            # ~1.4 us before a full-tile order would. DIRECT2D programs
            # cost ~0.7 us each on the issuing engine, so they all live
            # on Sync - ScalarE must start its ACT stream immediately.
            ats, bts = [], []
            for t in range(NTILES):
                at = io_pool.tile([P, 2 * D], f16, tag="a")
                bt = io_pool.tile([P, 2 * D], f16, tag="b")
                ats.append(at)
                bts.append(bt)

            def dma_piece(dst, src, t, j0, nj):
                base = t * RPT * D + j0 * D
                src_ap = bass.AP(
                    tensor=src, offset=base, ap=[[2 * D, P], [1, nj * D]]
                )
                nc.sync.dma_start(out=dst[:, j0 * D : (j0 + nj) * D], in_=src_ap)

            dma_piece(ats[0], a, 0, 0, 1)
            dma_piece(bts[0], b, 0, 0, 1)
            dma_piece(ats[0], a, 0, 1, 1)
            dma_piece(bts[0], b, 0, 1, 1)
            for t in range(1, NTILES):
                dma_piece(ats[t], a, t, 0, 2)
                dma_piece(bts[t], b, t, 0, 2)

            # --- constants -------------------------------------------
            # Sqrt warm first on ScalarE: forces the sqrt table set to
            # load during the DMA wait; Square (a cheap filler present
            # in every set) then needs no second load.
            ones = st_pool.tile([P, 1], f32)
            nc.vector.memset(ones, 1.0)
            warm = st_pool.tile([P, 1], f32)
            nc.scalar.sqrt(warm, ones)

            lab_sb = st_pool.tile([P, NBLK], f32)
            nc.scalar.dma_start(out=lab_sb, in_=lab[:, :])

            na = st_pool.tile([P, NBLK], f32)
            # dot_c / nb_c interleaved: col 2c = dot_c, col 2c+1 = nb_c
            # (the per-pair segmented reduce writes 4 columns in one op).
            stats_db = st_pool.tile([P, 2 * NBLK], f32)

            id128 = st_pool.tile([P, P], f16)
            make_identity(nc, id128)
            wsrc = st_pool.tile([P, 512], f16)
            nc.vector.memset(wsrc, 0.0)

            # PE warmup: keep the PE busy from the preamble so the HAM
            # activity window opens (2.4 GHz) before the first fold.
            wpsum = psa_pool.tile([P, 512], f32, tag="warm")
            for w in range(PE_WARM):
                nc.tensor.matmul(wpsum, id128, wsrc[:, :])

            # --- main loop: 8 blocks of 128 rows, paired for PSUM -----
            # Pair g = blocks (2g, 2g+1); PSUM bank [128, 4, 128] with
            # slots (dot_2g, nb_2g, dot_2g+1, nb_2g+1).
            fps = None
            for c in range(NBLK):
                t, j = divmod(c, 2)
                g, h = divmod(c, 2)
                asl = ats[t][:, j * D : (j + 1) * D]
                bsl = bts[t][:, j * D : (j + 1) * D]

                # ScalarE: na_c = sum a^2, fused square+row-accumulate.
                sa = sq_pool.tile([P, D], f16, tag="sq")
                nc.scalar.activation(
                    out=sa,
                    in_=asl,
                    func=mybir.ActivationFunctionType.Square,
                    accum_out=na[:, c : c + 1],
                )

                # Products in halves of one scratch tile so the PE fold
                # reads both with a single 3D AP. dot on VectorE
                # (2x_1P fp16); nb on GpSimd for GP_NB blocks.
                pt = prod_pool.tile([P, 2 * D], f16, tag="p")
                nc.vector.tensor_mul(pt[:, 0:D], asl, bsl)
                if c in GP_NB:
                    nc.gpsimd.tensor_mul(pt[:, D : 2 * D], bsl, bsl)
                else:
                    nc.vector.tensor_mul(pt[:, D : 2 * D], bsl, bsl)

                # TensorE: fold [P, 2048] -> PSUM slots [P, 2, 128].
                if h == 0:
                    fps = fold_pool.tile([P, 4, P], f32)
                pt4 = pt.rearrange("p (s k c) -> p s k c", s=2, k=KCH, c=P)
                for k in range(KCH):
                    nc.tensor.matmul(
                        fps[:, 2 * h : 2 * h + 2, :],
                        id128,
                        pt4[:, :, k, :],
                        start=(k == 0),
                        stop=(k == KCH - 1),
                    )

                if h == 1:
                    # VectorE: segmented reduce -> 4 stat columns.
                    nc.vector.tensor_reduce(
                        out=stats_db[:, 4 * g : 4 * g + 4],
                        in_=fps,
                        axis=mybir.AxisListType.X,
                        op=mybir.AluOpType.add,
                    )

            # --- tail ------------------------------------------------
            # score = dot / sqrt(na*nb); diff = score - label. Groups
            # 0-2 (blocks 0..5) batch into one pass once pair 2 lands
            # (off the critical path while tile 3 streams); group 3 is
            # the only tail work after the last block's stats.
            diff = st_pool.tile([P, 2 * NBLK], f32)  # cols 2c used
            for lo, hi in ((0, 6), (6, 8)):
                w = hi - lo
                cols = slice(2 * lo, 2 * hi, 2)
                nbv = stats_db[:, 2 * lo + 1 : 2 * hi : 2]
                nav = na[:, lo:hi]
                dv = stats_db[:, 2 * lo : 2 * hi : 2]
                pr = st_pool.tile([P, w], f32, tag=f"pr{lo}")
                nc.vector.tensor_mul(pr, nav, nbv)
                nc.scalar.sqrt(pr, pr)
                rs = st_pool.tile([P, w], f32, tag=f"rs{lo}")
                nc.vector.reciprocal(rs, pr)
                sc = st_pool.tile([P, w], f32, tag=f"sc{lo}")
                nc.vector.tensor_mul(sc, dv, rs)
                nc.vector.tensor_sub(diff[:, cols], sc, lab_sb[:, lo:hi])

            sqd = st_pool.tile([P, NBLK], f32)
            partial = st_pool.tile([P, 1], f32)
            nc.vector.scalar_tensor_tensor(
                out=sqd,
                in0=diff[:, 0 : 2 * NBLK : 2],
                scalar=1.0,
                in1=diff[:, 0 : 2 * NBLK : 2],
                op0=mybir.AluOpType.mult,
                op1=mybir.AluOpType.mult,
                accum_out=partial,
            )
            # Reduce 128 partitions -> [1,1] so the output DMA is one
            # descriptor instead of 128.
            total_ps = psa_pool.tile([1, 1], f32)
            nc.tensor.matmul(total_ps, partial, ones)
            res_sb = st_pool.tile([1, 1], f32)
            nc.scalar.copy(res_sb, total_ps)
            nc.sync.dma_start(out=out[:, :], in_=res_sb)

    nc.compile()
    return nc


def _label_perm(lab_core):
    """[ROWS] -> [P, NBLK] with labt[p, c] = labels[256*(c//2) + 2p + (c%2)],
    matching the stats layout (block c = 2t+j, partition p = row 2p+j)."""
    # reshape to [t, p, j] then arrange as [p, (t j)]
    return np.ascontiguousarray(
        lab_core.reshape(NTILES, P, 2).transpose(1, 0, 2).reshape(P, NBLK)
    )


def _prep_in_maps(issues_1_geb, issues_2_geb, labels):
    a16 = np.ascontiguousarray(issues_1_geb, dtype=np.float16)
    b16 = np.ascontiguousarray(issues_2_geb, dtype=np.float16)
    lab = np.ascontiguousarray(labels, dtype=np.float32)
    in_maps = []
    for c in range(N_CORES):
        sl = slice(c * ROWS, (c + 1) * ROWS)
        in_maps.append(
            {
                "a": np.ascontiguousarray(a16[sl]),
                "b": np.ascontiguousarray(b16[sl]),
                "lab_t": _label_perm(lab[sl]),
            }
        )
    return in_maps


def kernel(issues_1_geb, issues_2_geb, labels):
    if "nc" not in _cache:
        _cache["nc"] = _build()
    nc = _cache["nc"]

    in_maps = _prep_in_maps(issues_1_geb, issues_2_geb, labels)
    res = run_bass_kernel_spmd(nc, in_maps, core_ids=list(range(N_CORES)))
    total = np.float64(0.0)
    for r in res.results:
        total += np.float64(r["out"].sum(dtype=np.float64))
    return np.array(total / N, dtype=np.float32)
